# revision 20
# baseline (speedup 1.0000x reference)
"""Trainium2 Bass kernel for a CQT (constant-Q transform) nn.Module.

Reference computation (per batch sample b, channel c):
    out[b, c, k, f, 0] = sum_t x[b, c, f*HOP + t] * w_re[k, t]
    out[b, c, k, f, 1] = sum_t x[b, c, f*HOP + t] * w_im[k, t]
where w_re/w_im are Hann-windowed complex exponentials with per-bin ragged
lengths (longest 11340 samples), HOP=512, 84 bins, 409 frames.

Strategy: data-parallel over the batch (1 sample per NeuronCore, 8 cores).
Per core the PE matmuls put FRAMES on the output partition axis (stationary
operand = a 128-column slice of the resident signal tile) and the 168
interleaved (re,im) bin rows on the moving free axis.  Both channels'
frames are concatenated on one virtual frame axis; 7 frame-tiles of 128
cover all 2*409 frames (tile 3 straddles the channel seam, tile 6 has 72
live frames).  The contraction axis is split into 45 DoubleRow PAIRS of
K=256 samples each.

Precision scheme (all matmuls are fp8e4 DoubleRow at 0.5 cycles/row):
  - plain tier: per pair, the window-head region (whole pair below
    TH_LO*L, the longest bins = column prefix) and window-tail region
    (pair beyond TH_HI*L, the shortest active bins = column suffix) run a
    single w8*x8 matmul.  Covered window energy fraction eta gives a
    relative error ~0.027*sqrt(2*eta).
  - triple tier: the energetic window middles run THREE DoubleRow
    matmuls, w8*x8 + wr8*x8 + w8*xr8, where wr8/xr8 are the fp8-quantized
    quantization residuals of w and x.  This cancels both operands' fp8
    error to first order (remaining error ~sigma^2, negligible), at 1.5
    cycles/pair-column -- still cheaper than the 2.0 of an fp16 pair.
  There is no fp16 path at all, which also removes the fp16 signal/window
  DMA traffic.

Per stream (frame tile), phase A (all pairs' w8*x8 mid+prefix+suffix) is
emitted before phase B (residual matmuls), so early PE work only needs
the w8/x8 arrays whose DMA lands first; the Tile scheduler further
reorders across streams by readiness.  Pair 0's w8*x8 matmul is widened
to the full 168 columns so the stream's first write arms the whole PSUM
row (uniform pending-zero state).  PSUM accumulates in fp32, one bank
per frame-tile plus a warm-up scratch bank; dummy warm-up matmuls cover
the initial DMA latency.

Scheduling rules learned from the CoreSim cost model (v1), which the
whole DMA plan is built around:
  - A dma_start occupies its queue for max(500ns descriptor-gen,
    per-partition-bytes * 0.3855ns * (2 if contiguous element < 512B)).
  - A consumer whose semaphore wait is ALREADY satisfied when its engine
    reaches it proceeds free; a consumer that has to BLOCK on a DMA
    semaphore is released ~1716ns after the semaphore fires (the DMA
    pipe's init_delay).  Hence every piece is sized into [512, 648]
    fp8-cols so both the element-size and the transfer<=gen-window
    conditions hold, and the warm-up matmuls are tuned so the PE reaches
    the first real matmul a few ns AFTER the first pieces' semaphores
    (N_WARM*WARM_N is a cliff: too short -> +1716ns, K_WARMN=96 shows
    it).
  - Each queue's end-of-kernel drain costs last-DMA-slice-end + 1716ns
    unconditionally, so the kernel ends ~2.9us after the last output
    DMA slice no matter what; only PE-end time is worth optimizing.
  - The PE p-state ramp is wall-clock: matmuls before t=3us run at
    1.2GHz, after at 2.4GHz.
"""

import math
import os as _os
from contextlib import ExitStack

import ml_dtypes
import numpy as np

import concourse.bass as bass
import concourse.mybir as mybir
import concourse.tile as tile
from concourse import bacc
from concourse.bass_utils import run_bass_kernel_spmd

# ---- problem constants (hardcoded CQT spec) ----
SR = 22050
N_BINS = 84
BPO = 12
FMIN = 32.7
HOP = 512
B, C, T = 8, 2, 220500
N_CORES = 8

LMAX = 11340            # longest window
F = 409                 # frames: 1 + (T - LMAX)//HOP
NCHUNK = 90             # contraction chunks of 128 (padded to even count)
NPAIR = 45              # fp8 DoubleRow chunk pairs
NROWS = 2 * N_BINS      # interleaved (re, im) weight rows
MBLK = 431              # 512-sample blocks per channel (ceil(220500/512))
MB2 = 2 * MBLK          # concatenated block axis (ch0 | ch1)
MB2P = 1152             # x8 inner width: padded so each half-tile DMA piece
                        # is 576 cols (>=512B contiguous elem and transfer
                        # under the 500ns descriptor-gen floor -- both needed
                        # for the fast DMA-semaphore path in the cost model);
                        # fp8 dual-row Ldweights needs plane stride % 4 == 0
NTILE = 7               # frame tiles of 128 over the 840-virtual-frame axis
V0 = [0, 128, 256, 384, 512, 640, 768]
MT = [128, 128, 128, 128, 128, 128, 72]  # live partition count per tile

F8_DT = mybir.dt.float8e4
OUT_DT = mybir.dt.float16  # staging/output dtype (host casts back to f32)
F8_NP = ml_dtypes.float8_e4m3
MM_DT = mybir.dt.float16   # warm-up matmul dtype

TH_LO = float(_os.environ.get("K_THLO", "0.345"))  # plain-fp8 window-start
TH_HI = float(_os.environ.get("K_THHI", "0.635"))  # plain-fp8 window-tail
TRUNC = float(_os.environ.get("K_TRUNC", "1.0"))  # drop window tail past T*L
N_WARM = int(_os.environ.get("K_NWARM", "3"))    # warm-up matmuls
WARM_N = int(_os.environ.get("K_WARMN", "112"))  # their moving size

_PREP = None
_NC = None
LAST_RESULTS = None


def _params():
    """Host-side constants: pair geometry, tier selection, and the w8
    weight array laid out in emission order (phase-A blocks then phase-B
    residual blocks)."""
    global _PREP
    if _PREP is not None:
        return _PREP

    Q = 1.0 / (2.0 ** (1.0 / BPO) - 1.0)
    freqs = FMIN * 2.0 ** (np.arange(N_BINS, dtype=np.float64) / BPO)
    lengths = np.round(Q * SR / freqs).astype(np.int64)
    assert int(lengths.max()) == LMAX

    lengths_eff = np.round(lengths * TRUNC).astype(np.int64)
    t = np.arange(LMAX, dtype=np.float64)
    L = lengths.astype(np.float64)[:, None]
    mask = (t[None, :] < lengths_eff[:, None]).astype(np.float64)
    win = 0.5 * (1.0 - np.cos(2.0 * math.pi * t[None, :] / L)) * mask
    phase = (2.0 * math.pi / SR) * freqs[:, None] * t[None, :]
    w_re = (win * np.cos(phase)).astype(np.float32)
    w_im = (-win * np.sin(phase)).astype(np.float32)

    # rows 2k / 2k+1 = re_k / im_k; zero-pad time to NCHUNK*128
    W = np.zeros((NROWS, NCHUNK * 128), dtype=np.float32)
    W[0::2, :LMAX] = w_re
    W[1::2, :LMAX] = w_im
    WT = np.ascontiguousarray(W.T)  # (NCHUNK*128, NROWS)
    W8full = WT.astype(F8_NP)                       # fp8(w)
    WRfull = (WT - W8full.astype(np.float32)).astype(F8_NP)  # fp8(w - fp8(w))

    mcols = np.array([2 * int((lengths_eff > 128 * c).sum())
                      for c in range(NCHUNK)], dtype=np.int64)
    assert mcols[0] == NROWS

    # tier selection per pair q (samples [256q, 256(q+1))):
    #   prefix cols [0:p8): longest bins, pair within window-head region
    #   suffix cols [s8:mcols[2q]): shortest active bins, pair in tail
    #   mid cols [p8:s8): triple tier
    p8 = np.zeros(NPAIR, dtype=np.int64)
    s8 = np.zeros(NPAIR, dtype=np.int64)
    for q in range(NPAIR):
        lo, hi = 256 * q, 256 * (q + 1)
        p8[q] = 2 * int((lengths >= hi / TH_LO).sum()) if TH_LO > 0 else 0
        n_not_suf = int((lengths * TH_HI > lo).sum())
        s8[q] = 2 * max(n_not_suf, p8[q] // 2)
        s8[q] = min(s8[q], mcols[2 * q])
        p8[q] = min(p8[q], s8[q])

    # emission blocks. Phase A: per pair, MIDW (w8*x8; pair 0 widened to
    # the full active width for PSUM arming), PRE, SUF. Phase B: per
    # pair, MIDR (wr8*x8) and the same-range w8*xr8 (reuses MIDW's
    # columns via offset arithmetic).
    def mid_range(q):
        return (int(p8[q]), int(s8[q]))

    arm_via_memset = int(_os.environ.get("K_ARM", "0"))

    def midw_range(q):
        if q == 0 and not arm_via_memset:
            return (0, int(mcols[0]))
        return mid_range(q)

    woff = {}
    off = 0
    for q in range(NPAIR):          # phase A blocks
        lo, hi = midw_range(q)
        if hi > lo:
            woff[("midw", q)] = off
            off += hi - lo
        if p8[q] > 0 and q != 0:
            woff[("pre", q)] = off
            off += int(p8[q])
        if mcols[2 * q] > s8[q]:
            woff[("suf", q)] = off
            off += int(mcols[2 * q] - s8[q])
    SA = int(off)
    for q in range(NPAIR):          # phase B blocks (residual weights)
        lo, hi = mid_range(q)
        if hi > lo:
            woff[("midr", q)] = off
            off += hi - lo
    S = int(off + (-off) % 4)  # fp8 dual-row plane stride must be 4-aligned

    w8 = np.zeros((128, 2, S), dtype=np.float32)
    for key, o in woff.items():
        kind, q = key
        if kind == "midw":
            lo, hi = midw_range(q)
            src = W8full
        elif kind == "pre":
            lo, hi = 0, int(p8[q])
            src = W8full
        elif kind == "suf":
            lo, hi = int(s8[q]), int(mcols[2 * q])
            src = W8full
        else:  # midr
            lo, hi = mid_range(q)
            src = WRfull
        n = hi - lo
        for i in range(2):
            blk = src[128 * (2 * q + i):128 * (2 * q + i + 1), lo:hi]
            w8[:, i, o:o + n] = blk.astype(np.float32)
    w8 = w8.astype(F8_NP)

    _PREP = dict(mcols=mcols, p8=p8, s8=s8, mid_range=mid_range,
                 midw_range=midw_range, woff=woff, SA=SA, S=S, w8=w8,
                 arm_via_memset=arm_via_memset)
    return _PREP


def _dma_plan(p):
    """(queue, tensor, [u,] lo, hi) pieces, in per-queue emission order.

    Every piece's contiguous element must be >= 512 bytes: the CoreSim
    DMA model delays a sub-512B-element DMA's semaphore by ~1.7us, which
    stalls every consumer.  x8/xr8 go as full-tile DMAs (contiguous
    2*MB2P bytes per partition); w8 pieces are cut at block boundaries
    at least 512 columns apart.
    """
    woff, SA, S = p["woff"], p["SA"], p["S"]
    a_bounds = sorted(o for o in
                      [woff.get(("midw", q), None) for q in range(NPAIR)]
                      if o is not None)
    def cut(target, bounds):
        c = min(bounds, key=lambda o: abs(o - target))
        return c
    # piece size must be in [512, 648] cols: >=512B contiguous element AND
    # transfer (2 planes * cols * 0.3855ns) within the 500ns descriptor-gen
    # window -- both required for the fast DMA-semaphore path
    ncut = max(1, round(SA / 580))
    cut_adj = int(_os.environ.get("K_CUT", "40"))
    cuts = [0]
    for i in range(1, ncut):
        c = cut(SA * i // ncut + cut_adj, a_bounds)
        if c - cuts[-1] >= 512 and SA - c >= 512:
            cuts.append(c)
    cuts.append(SA)
    a_pieces = [(cuts[i], cuts[i + 1]) for i in range(len(cuts) - 1)]
    b_bounds = sorted(o for o in
                      [woff.get(("midr", q), None) for q in range(NPAIR)]
                      if o is not None) + [S]
    b0 = b_bounds[0]
    for lo, hi in a_pieces:
        assert hi - lo >= 512, (a_pieces,)
    assert S - b0 >= 512, (b0, S)
    a1 = a_pieces[0][1] if len(a_pieces) > 1 else SA
    a2 = a_pieces[1][1] if len(a_pieces) > 2 else SA
    plan = int(_os.environ.get("K_PLAN", "0"))

    if plan == 0:  # wave order: queues deliver in consumption priority
        w_pieces = ([("sync", "w8", lo, hi) for lo, hi in a_pieces]
                    + [("sync", "w8", b0, S)])
        x_pieces = [
            ("scalar", "x8", 0, 0, 576),
            ("scalar", "x8", 0, 576, MB2P),
            ("scalar", "xr8", 0, 0, 576),
            ("scalar", "xr8", 0, 576, MB2P),
            ("gpsimd", "x8", 1, 0, 576),
            ("gpsimd", "x8", 1, 576, MB2P),
            ("gpsimd", "xr8", 1, 0, 576),
            ("gpsimd", "xr8", 1, 576, MB2P),
        ]
        out_qs = ["scalar", "gpsimd", "sync", "scalar", "gpsimd", "scalar",
                  "sync"]
    elif plan == 1:  # all w8 on sync, residuals after A
        w_pieces = ([("sync", "w8", lo, hi) for lo, hi in a_pieces]
                    + [("sync", "w8", b0, S)])
        x_pieces = [
            ("scalar", "x8", 0, 0, 576),
            ("scalar", "x8", 0, 576, MB2P),
            ("scalar", "xr8", 0, 0, 576),
            ("scalar", "xr8", 0, 576, MB2P),
            ("gpsimd", "x8", 1, 0, 576),
            ("gpsimd", "x8", 1, 576, MB2P),
            ("gpsimd", "xr8", 1, 0, 576),
            ("gpsimd", "xr8", 1, 576, MB2P),
        ]
        out_qs = ["scalar", "gpsimd", "sync", "scalar", "gpsimd", "scalar",
                  "sync"]
    elif plan == 3:  # plan 0 with u roles swapped (u0 on gpsimd)
        w_pieces = ([("sync", "w8", lo, hi) for lo, hi in a_pieces]
                    + [("sync", "w8", b0, S)])
        x_pieces = [
            ("gpsimd", "x8", 0, 0, 576),
            ("gpsimd", "x8", 0, 576, MB2P),
            ("gpsimd", "xr8", 0, 0, 576),
            ("gpsimd", "xr8", 0, 576, MB2P),
            ("scalar", "x8", 1, 0, 576),
            ("scalar", "x8", 1, 576, MB2P),
            ("scalar", "xr8", 1, 0, 576),
            ("scalar", "xr8", 1, 576, MB2P),
        ]
        out_qs = ["scalar", "gpsimd", "sync", "scalar", "gpsimd", "scalar",
                  "sync"]
    else:  # plan 2: residual weights between A pieces on sync
        w_pieces = ([("sync", "w8", lo, hi)
                     for lo, hi in a_pieces[:max(1, len(a_pieces) - 1)]]
                    + [("sync", "w8", b0, S)]
                    + [("sync", "w8", lo, hi) for lo, hi in a_pieces[max(1, len(a_pieces) - 1):]])
        x_pieces = [
            ("scalar", "x8", 0, 0, MB2P),
            ("scalar", "xr8", 0, 0, MB2P),
            ("gpsimd", "x8", 1, 0, MB2P),
            ("gpsimd", "xr8", 1, 0, MB2P),
        ]
        out_qs = ["scalar", "gpsimd", "sync", "scalar", "gpsimd", "scalar",
                  "sync"]
    return w_pieces, x_pieces, out_qs


def _build_nc():
    p = _params()
    mcols, p8, s8 = p["mcols"], p["p8"], p["s8"]
    mid_range, midw_range, woff = p["mid_range"], p["midw_range"], p["woff"]
    S = p["S"]
    w_pieces, x_pieces, out_qs = _dma_plan(p)

    nc = bacc.Bacc(None, target_bir_lowering=False)
    x8_d = nc.dram_tensor("x8", (2, 128, 2, MB2P), F8_DT, kind="ExternalInput")
    xr8_d = nc.dram_tensor("xr8", (2, 128, 2, MB2P), F8_DT,
                           kind="ExternalInput")
    w8_d = nc.dram_tensor("w8", (128, 2, S), F8_DT, kind="ExternalInput")
    out_d = nc.dram_tensor("out", (NTILE, 128, NROWS), OUT_DT,
                           kind="ExternalOutput")

    with ExitStack() as ctx:
        tc = ctx.enter_context(tile.TileContext(nc))
        xp = ctx.enter_context(tc.tile_pool(name="xp", bufs=1))
        wp = ctx.enter_context(tc.tile_pool(name="wp", bufs=1))
        op = ctx.enter_context(tc.tile_pool(name="op", bufs=1))
        pp = ctx.enter_context(tc.tile_pool(name="pp", bufs=1, space="PSUM"))

        # PSUM: one full bank per frame-tile + one warm-up scratch bank
        ps = [pp.tile([128, 512], mybir.dt.float32, name=f"ps{t}",
                      tag=f"ps{t}") for t in range(NTILE)]
        pw = pp.tile([128, 512], mybir.dt.float32, name="pw", tag="pw")

        warm_sb = xp.tile([128, max(WARM_N, 128)], MM_DT, name="warm",
                          tag="warm")
        nc.vector.memset(warm_sb[:].bitcast(mybir.dt.float32), 0.0)
        if int(_os.environ.get("K_ARM", "0")):
            for t in range(NTILE):
                nc.vector.memset(ps[t][:, 0:NROWS], 0.0)
        for _ in range(N_WARM):
            nc.tensor.matmul(pw[:, 0:WARM_N], warm_sb[:, 0:128],
                             warm_sb[:, 0:WARM_N],
                             start=True, stop=True, skip_group_check=True)

        # --- SBUF tiles + input DMA streams ---
        x8s = [xp.tile([128, 2, MB2P], F8_DT, name=f"x8_{u}", tag=f"x8_{u}")
               for u in range(2)]
        xr8s = [xp.tile([128, 2, MB2P], F8_DT, name=f"xr8_{u}",
                        tag=f"xr8_{u}") for u in range(2)]
        w8s = wp.tile([128, 2, S], F8_DT, name="w8_sb", tag="w8_sb")

        qs = {"sync": nc.sync, "scalar": nc.scalar, "gpsimd": nc.gpsimd,
              "vector": nc.vector}
        for q, tn, lo, hi in w_pieces:
            if hi > lo:
                qs[q].dma_start(w8s[:, :, lo:hi], w8_d[:, :, lo:hi])
        for q, tn, u, lo, hi in x_pieces:
            src = x8_d if tn == "x8" else xr8_d
            dst = x8s if tn == "x8" else xr8s
            qs[q].dma_start(dst[u][:, :, lo:hi], src[u][:, :, lo:hi])

        # --- matmul streams: one per frame-tile ---
        DR = mybir.MatmulPerfMode.DoubleRow

        def stream_emits(qlist_a, qlist_b):
            emits = []
            for q in qlist_a:           # phase A
                lo, hi = midw_range(q)
                if hi > lo:
                    emits.append(("midw", q, lo, hi, woff[("midw", q)]))
                if p8[q] > 0 and q != 0:
                    emits.append(("pre", q, 0, int(p8[q]),
                                  woff[("pre", q)]))
                if mcols[2 * q] > s8[q]:
                    emits.append(("suf", q, int(s8[q]), int(mcols[2 * q]),
                                  woff[("suf", q)]))
            for q in qlist_b:           # phase B
                lo, hi = mid_range(q)
                if hi > lo:
                    emits.append(("midr", q, lo, hi, woff[("midr", q)]))
                    # w8 * xr8 reuses the MIDW block columns
                    wlo, _ = midw_range(q)
                    emits.append(("xres", q, lo, hi,
                                  woff[("midw", q)] + (lo - wlo)))
            return emits

        def emit_mms(t, emits, first, last):
            v0, m = V0[t], MT[t]
            for n, (kind, q, lo, hi, o8) in enumerate(emits):
                u, jp = q % 2, q // 2
                stat = xr8s[u] if kind == "xres" else x8s[u]
                nc.tensor.matmul(
                    ps[t][0:m, lo:hi],
                    stat[:, :, v0 + jp:v0 + jp + m],
                    w8s[:, :, o8:o8 + hi - lo],
                    perf_mode=DR,
                    start=(first and n == 0),
                    stop=(last and n == len(emits) - 1),
                    skip_group_check=True)

        arm = p["arm_via_memset"]

        def emit_stream(t):
            emit_mms(t, stream_emits(range(NPAIR), range(NPAIR)),
                     not arm, True)

        ots = [op.tile([128, NROWS], OUT_DT, name=f"o{t}", tag=f"o{t}")
               for t in range(NTILE)]
        tail_split = int(_os.environ.get("K_TAILSPLIT", "0"))
        # bins 42..83 (cols 84:168) have windows <= pair 3; after pairs 0-3
        # their PSUM columns are final and can be copied mid-stream
        for t in range(NTILE):
            m = MT[t]
            if tail_split and t == NTILE - 1:
                emit_mms(t, stream_emits(range(4), range(4)), True, False)
                nc.vector.tensor_copy(ots[t][0:m, 84:NROWS],
                                      ps[t][0:m, 84:NROWS])
                emit_mms(t, stream_emits(range(4, NPAIR), range(4, NPAIR)),
                         False, True)
                nc.vector.tensor_copy(ots[t][0:m, 0:84], ps[t][0:m, 0:84])
            else:
                emit_stream(t)
                nc.vector.tensor_copy(ots[t][0:m, :], ps[t][0:m, 0:NROWS])
            qs[out_qs[t]].dma_start(out_d[t, 0:m, :], ots[t][0:m, :])
    nc.finalize()
    return nc


def get_nc():
    global _NC
    if _NC is None:
        _NC = _build_nc()
    return _NC


def _pack_x(xb):
    """(C, T) -> fp8 x8 (2, 128, 2, MB2P) + fp8 residual xr8 (same).

    x8[u, r, i, m] = fp8(xcat[m*512 + u*256 + i*128 + r]);
    xr8 = fp8(xcat - x8).  xcat = [ch0 blocks | ch1 blocks], zero tails."""
    xpad = np.zeros((C, MBLK * 512), dtype=np.float32)
    xpad[:, :T] = xb
    xcat = xpad.reshape(MB2, 512)
    xq = xcat.astype(F8_NP)
    xr = (xcat - xq.astype(np.float32)).astype(F8_NP)
    x8 = np.zeros((2, 128, 2, MB2P), dtype=F8_NP)
    xr8 = np.zeros((2, 128, 2, MB2P), dtype=F8_NP)
    x8[:, :, :, :MB2] = np.ascontiguousarray(
        xq.view(np.uint8).reshape(MB2, 2, 2, 128).transpose(1, 3, 2, 0)
    ).view(F8_NP)
    xr8[:, :, :, :MB2] = np.ascontiguousarray(
        xr.view(np.uint8).reshape(MB2, 2, 2, 128).transpose(1, 3, 2, 0)
    ).view(F8_NP)
    return x8, xr8


def _core_inputs(xb):
    p = _params()
    x8, xr8 = _pack_x(xb)
    return {"x8": x8, "xr8": xr8, "w8": p["w8"]}


def kernel(x):
    global LAST_RESULTS
    x = np.asarray(x, dtype=np.float32)
    assert x.shape == (B, C, T)
    in_maps = [_core_inputs(x[b]) for b in range(B)]
    nc = get_nc()
    res = run_bass_kernel_spmd(nc, in_maps, core_ids=list(range(N_CORES)))
    LAST_RESULTS = res
    out = np.empty((B, C, N_BINS, F, 2), dtype=np.float32)
    for b in range(B):
        raw = np.asarray(res.results[b]["out"])  # (NTILE, 128, NROWS)
        out[b] = _unpack_out(raw)
    return out


def _unpack_out(raw):
    """(NTILE, 128, NROWS) -> (C, N_BINS, F, 2)."""
    raw = np.asarray(raw, dtype=np.float32)
    cat = raw.reshape(NTILE * 128, NROWS)[:V0[-1] + MT[-1]]  # (840, 168)
    o = np.empty((C, N_BINS, F, 2), dtype=np.float32)
    o[0] = cat[0:F].reshape(F, N_BINS, 2).transpose(1, 0, 2)
    o[1] = cat[MBLK:MBLK + F].reshape(F, N_BINS, 2).transpose(1, 0, 2)
    return o


# revision 22
# speedup vs baseline: 1.0249x; 1.0249x over previous
"""Trainium2 Bass kernel for a CQT (constant-Q transform) nn.Module.

Reference computation (per batch sample b, channel c):
    out[b, c, k, f, 0] = sum_t x[b, c, f*HOP + t] * w_re[k, t]
    out[b, c, k, f, 1] = sum_t x[b, c, f*HOP + t] * w_im[k, t]
where w_re/w_im are Hann-windowed complex exponentials with per-bin ragged
lengths (longest 11340 samples), HOP=512, 84 bins, 409 frames.

Strategy: data-parallel over the batch (1 sample per NeuronCore, 8 cores).
Per core the PE matmuls put FRAMES on the output partition axis (stationary
operand = a 128-column slice of the resident signal tile) and the 168
interleaved (re,im) bin rows on the moving free axis.  Both channels'
frames are concatenated on one virtual frame axis; 7 frame-tiles of 128
cover all 2*409 frames (tile 3 straddles the channel seam, tile 6 has 72
live frames).  The contraction axis is split into 45 DoubleRow PAIRS of
K=256 samples each.

Precision scheme (all matmuls are fp8e4 DoubleRow at 0.5 cycles/row):
  - plain tier: per pair, the window-head region (whole pair below
    TH_LO*L, the longest bins = column prefix) and window-tail region
    (pair beyond TH_HI*L, the shortest active bins = column suffix) run a
    single w8*x8 matmul.  Covered window energy fraction eta gives a
    relative error ~0.027*sqrt(2*eta).
  - triple tier: the energetic window middles run THREE DoubleRow
    matmuls, w8*x8 + wr8*x8 + w8*xr8, where wr8/xr8 are the fp8-quantized
    quantization residuals of w and x.  This cancels both operands' fp8
    error to first order (remaining error ~sigma^2, negligible), at 1.5
    cycles/pair-column -- still cheaper than the 2.0 of an fp16 pair.
  There is no fp16 path at all, which also removes the fp16 signal/window
  DMA traffic.

Per stream (frame tile), phase A (all pairs' w8*x8 mid+prefix+suffix) is
emitted before phase B (residual matmuls), so early PE work only needs
the w8/x8 arrays whose DMA lands first; the Tile scheduler further
reorders across streams by readiness.  Pair 0's w8*x8 matmul is widened
to the full 168 columns so the stream's first write arms the whole PSUM
row (uniform pending-zero state).  PSUM accumulates in fp32, one bank
per frame-tile plus a warm-up scratch bank; dummy warm-up matmuls cover
the initial DMA latency.

Scheduling rules learned from the CoreSim cost model (v1), which the
whole DMA plan is built around:
  - A dma_start occupies its queue for max(500ns descriptor-gen,
    per-partition-bytes * 0.3855ns * (2 if contiguous element < 512B)).
  - A consumer whose semaphore wait is ALREADY satisfied when its engine
    reaches it proceeds free; a consumer that has to BLOCK on a DMA
    semaphore is released ~1716ns after the semaphore fires (the DMA
    pipe's init_delay).  Hence every piece is sized into [512, 648]
    fp8-cols so both the element-size and the transfer<=gen-window
    conditions hold, and the warm-up matmuls are tuned so the PE reaches
    the first real matmul a few ns AFTER the first pieces' semaphores
    (N_WARM*WARM_N is a cliff: too short -> +1716ns, K_WARMN=96 shows
    it).
  - Each queue's end-of-kernel drain costs last-DMA-slice-end + 1716ns
    unconditionally, so the kernel ends ~2.9us after the last output
    DMA slice no matter what; only PE-end time is worth optimizing.
  - The PE p-state ramp is wall-clock: matmuls before t=3us run at
    1.2GHz, after at 2.4GHz.
"""

import math
import os as _os
from contextlib import ExitStack

import ml_dtypes
import numpy as np

import concourse.bass as bass
import concourse.mybir as mybir
import concourse.tile as tile
from concourse import bacc
from concourse.bass_utils import run_bass_kernel_spmd

# ---- problem constants (hardcoded CQT spec) ----
SR = 22050
N_BINS = 84
BPO = 12
FMIN = 32.7
HOP = 512
B, C, T = 8, 2, 220500
N_CORES = 8

LMAX = 11340            # longest window
F = 409                 # frames: 1 + (T - LMAX)//HOP
NCHUNK = 90             # contraction chunks of 128 (padded to even count)
NPAIR = 45              # fp8 DoubleRow chunk pairs
NROWS = 2 * N_BINS      # interleaved (re, im) weight rows
MBLK = 431              # 512-sample blocks per channel (ceil(220500/512))
MB2 = 2 * MBLK          # concatenated block axis (ch0 | ch1)
MB2P = 1152             # x8 inner width: padded so each half-tile DMA piece
                        # is 576 cols (>=512B contiguous elem and transfer
                        # under the 500ns descriptor-gen floor -- both needed
                        # for the fast DMA-semaphore path in the cost model);
                        # fp8 dual-row Ldweights needs plane stride % 4 == 0
NTILE = 7               # frame tiles of 128 over the 840-virtual-frame axis
V0 = [0, 128, 256, 384, 512, 640, 768]
MT = [128, 128, 128, 128, 128, 128, 72]  # live partition count per tile

F8_DT = mybir.dt.float8e4
OUT_DT = mybir.dt.float16  # staging/output dtype (host casts back to f32)
F8_NP = ml_dtypes.float8_e4m3
MM_DT = mybir.dt.float16   # warm-up matmul dtype

TH_LO = float(_os.environ.get("K_THLO", "0.345"))  # plain-fp8 window-start
TH_HI = float(_os.environ.get("K_THHI", "0.635"))  # plain-fp8 window-tail
TRUNC = float(_os.environ.get("K_TRUNC", "1.0"))  # drop window tail past T*L
N_WARM = int(_os.environ.get("K_NWARM", "3"))    # warm-up matmuls
WARM_N = int(_os.environ.get("K_WARMN", "112"))  # their moving size

_PREP = None
_NC = None
LAST_RESULTS = None


def _params():
    """Host-side constants: pair geometry, tier selection, and the w8
    weight array laid out in emission order (phase-A blocks then phase-B
    residual blocks)."""
    global _PREP
    if _PREP is not None:
        return _PREP

    Q = 1.0 / (2.0 ** (1.0 / BPO) - 1.0)
    freqs = FMIN * 2.0 ** (np.arange(N_BINS, dtype=np.float64) / BPO)
    lengths = np.round(Q * SR / freqs).astype(np.int64)
    assert int(lengths.max()) == LMAX

    lengths_eff = np.round(lengths * TRUNC).astype(np.int64)
    t = np.arange(LMAX, dtype=np.float64)
    L = lengths.astype(np.float64)[:, None]
    mask = (t[None, :] < lengths_eff[:, None]).astype(np.float64)
    win = 0.5 * (1.0 - np.cos(2.0 * math.pi * t[None, :] / L)) * mask
    phase = (2.0 * math.pi / SR) * freqs[:, None] * t[None, :]
    w_re = (win * np.cos(phase)).astype(np.float32)
    w_im = (-win * np.sin(phase)).astype(np.float32)

    # rows 2k / 2k+1 = re_k / im_k; zero-pad time to NCHUNK*128
    W = np.zeros((NROWS, NCHUNK * 128), dtype=np.float32)
    W[0::2, :LMAX] = w_re
    W[1::2, :LMAX] = w_im
    WT = np.ascontiguousarray(W.T)  # (NCHUNK*128, NROWS)
    W8full = WT.astype(F8_NP)                       # fp8(w)
    WRfull = (WT - W8full.astype(np.float32)).astype(F8_NP)  # fp8(w - fp8(w))

    mcols = np.array([2 * int((lengths_eff > 128 * c).sum())
                      for c in range(NCHUNK)], dtype=np.int64)
    assert mcols[0] == NROWS

    # tier selection per pair q (samples [256q, 256(q+1))):
    #   prefix cols [0:p8): longest bins, pair within window-head region
    #   suffix cols [s8:mcols[2q]): shortest active bins, pair in tail
    #   mid cols [p8:s8): triple tier
    p8 = np.zeros(NPAIR, dtype=np.int64)
    s8 = np.zeros(NPAIR, dtype=np.int64)
    for q in range(NPAIR):
        lo, hi = 256 * q, 256 * (q + 1)
        p8[q] = 2 * int((lengths >= hi / TH_LO).sum()) if TH_LO > 0 else 0
        n_not_suf = int((lengths * TH_HI > lo).sum())
        s8[q] = 2 * max(n_not_suf, p8[q] // 2)
        s8[q] = min(s8[q], mcols[2 * q])
        p8[q] = min(p8[q], s8[q])

    # emission blocks. Phase A: per pair, MIDW (w8*x8; pair 0 widened to
    # the full active width for PSUM arming), PRE, SUF. Phase B: per
    # pair, MIDR (wr8*x8) and the same-range w8*xr8 (reuses MIDW's
    # columns via offset arithmetic).
    def mid_range(q):
        return (int(p8[q]), int(s8[q]))

    arm_via_memset = int(_os.environ.get("K_ARM", "0"))

    def midw_range(q):
        if q == 0 and not arm_via_memset:
            return (0, int(mcols[0]))
        return mid_range(q)

    woff = {}
    off = 0
    for q in range(NPAIR):          # phase A blocks
        lo, hi = midw_range(q)
        if hi > lo:
            woff[("midw", q)] = off
            off += hi - lo
        if p8[q] > 0 and q != 0:
            woff[("pre", q)] = off
            off += int(p8[q])
        if mcols[2 * q] > s8[q]:
            woff[("suf", q)] = off
            off += int(mcols[2 * q] - s8[q])
    SA = int(off)
    for q in range(NPAIR):          # phase B blocks (residual weights)
        lo, hi = mid_range(q)
        if hi > lo:
            woff[("midr", q)] = off
            off += hi - lo
    S = int(off + (-off) % 4)  # fp8 dual-row plane stride must be 4-aligned

    w8 = np.zeros((128, 2, S), dtype=np.float32)
    for key, o in woff.items():
        kind, q = key
        if kind == "midw":
            lo, hi = midw_range(q)
            src = W8full
        elif kind == "pre":
            lo, hi = 0, int(p8[q])
            src = W8full
        elif kind == "suf":
            lo, hi = int(s8[q]), int(mcols[2 * q])
            src = W8full
        else:  # midr
            lo, hi = mid_range(q)
            src = WRfull
        n = hi - lo
        for i in range(2):
            blk = src[128 * (2 * q + i):128 * (2 * q + i + 1), lo:hi]
            w8[:, i, o:o + n] = blk.astype(np.float32)
    w8 = w8.astype(F8_NP)

    _PREP = dict(mcols=mcols, p8=p8, s8=s8, mid_range=mid_range,
                 midw_range=midw_range, woff=woff, SA=SA, S=S, w8=w8,
                 arm_via_memset=arm_via_memset)
    return _PREP


def _dma_plan(p):
    """(queue, tensor, [u,] lo, hi) pieces, in per-queue emission order.

    Every piece's contiguous element must be >= 512 bytes: the CoreSim
    DMA model delays a sub-512B-element DMA's semaphore by ~1.7us, which
    stalls every consumer.  x8/xr8 go as full-tile DMAs (contiguous
    2*MB2P bytes per partition); w8 pieces are cut at block boundaries
    at least 512 columns apart.
    """
    woff, SA, S = p["woff"], p["SA"], p["S"]
    a_bounds = sorted(o for o in
                      [woff.get(("midw", q), None) for q in range(NPAIR)]
                      if o is not None)
    def cut(target, bounds):
        c = min(bounds, key=lambda o: abs(o - target))
        return c
    # piece size must be in [512, 648] cols: >=512B contiguous element AND
    # transfer (2 planes * cols * 0.3855ns) within the 500ns descriptor-gen
    # window -- both required for the fast DMA-semaphore path
    ncut = max(1, round(SA / 580))
    cut_adj = int(_os.environ.get("K_CUT", "40"))
    cuts = [0]
    for i in range(1, ncut):
        c = cut(SA * i // ncut + cut_adj, a_bounds)
        if c - cuts[-1] >= 512 and SA - c >= 512:
            cuts.append(c)
    cuts.append(SA)
    a_pieces = [(cuts[i], cuts[i + 1]) for i in range(len(cuts) - 1)]
    b_bounds = sorted(o for o in
                      [woff.get(("midr", q), None) for q in range(NPAIR)]
                      if o is not None) + [S]
    b0 = b_bounds[0]
    for lo, hi in a_pieces:
        assert hi - lo >= 512, (a_pieces,)
    assert S - b0 >= 512, (b0, S)
    a1 = a_pieces[0][1] if len(a_pieces) > 1 else SA
    a2 = a_pieces[1][1] if len(a_pieces) > 2 else SA
    plan = int(_os.environ.get("K_PLAN", "0"))

    if plan == 0:  # wave order: queues deliver in consumption priority
        w_pieces = ([("sync", "w8", lo, hi) for lo, hi in a_pieces]
                    + [("sync", "w8", b0, S)])
        x_pieces = [
            ("scalar", "x8", 0, 0, 576),
            ("scalar", "x8", 0, 576, MB2P),
            ("scalar", "xr8", 0, 0, 576),
            ("scalar", "xr8", 0, 576, MB2P),
            ("gpsimd", "x8", 1, 0, 576),
            ("gpsimd", "x8", 1, 576, MB2P),
            ("gpsimd", "xr8", 1, 0, 576),
            ("gpsimd", "xr8", 1, 576, MB2P),
        ]
        oq = _os.environ.get("K_OUTQ", "")
        if oq:
            m = {"s": "scalar", "g": "gpsimd", "y": "sync"}
            out_qs = [m[c] for c in oq]
        else:
            out_qs = ["sync", "scalar", "gpsimd", "sync", "gpsimd",
                      "scalar", "scalar"]
    elif plan == 1:  # all w8 on sync, residuals after A
        w_pieces = ([("sync", "w8", lo, hi) for lo, hi in a_pieces]
                    + [("sync", "w8", b0, S)])
        x_pieces = [
            ("scalar", "x8", 0, 0, 576),
            ("scalar", "x8", 0, 576, MB2P),
            ("scalar", "xr8", 0, 0, 576),
            ("scalar", "xr8", 0, 576, MB2P),
            ("gpsimd", "x8", 1, 0, 576),
            ("gpsimd", "x8", 1, 576, MB2P),
            ("gpsimd", "xr8", 1, 0, 576),
            ("gpsimd", "xr8", 1, 576, MB2P),
        ]
        out_qs = ["scalar", "gpsimd", "sync", "scalar", "gpsimd", "scalar",
                  "sync"]
    elif plan == 3:  # plan 0 with u roles swapped (u0 on gpsimd)
        w_pieces = ([("sync", "w8", lo, hi) for lo, hi in a_pieces]
                    + [("sync", "w8", b0, S)])
        x_pieces = [
            ("gpsimd", "x8", 0, 0, 576),
            ("gpsimd", "x8", 0, 576, MB2P),
            ("gpsimd", "xr8", 0, 0, 576),
            ("gpsimd", "xr8", 0, 576, MB2P),
            ("scalar", "x8", 1, 0, 576),
            ("scalar", "x8", 1, 576, MB2P),
            ("scalar", "xr8", 1, 0, 576),
            ("scalar", "xr8", 1, 576, MB2P),
        ]
        out_qs = ["scalar", "gpsimd", "sync", "scalar", "gpsimd", "scalar",
                  "sync"]
    else:  # plan 2: residual weights between A pieces on sync
        w_pieces = ([("sync", "w8", lo, hi)
                     for lo, hi in a_pieces[:max(1, len(a_pieces) - 1)]]
                    + [("sync", "w8", b0, S)]
                    + [("sync", "w8", lo, hi) for lo, hi in a_pieces[max(1, len(a_pieces) - 1):]])
        x_pieces = [
            ("scalar", "x8", 0, 0, MB2P),
            ("scalar", "xr8", 0, 0, MB2P),
            ("gpsimd", "x8", 1, 0, MB2P),
            ("gpsimd", "xr8", 1, 0, MB2P),
        ]
        out_qs = ["scalar", "gpsimd", "sync", "scalar", "gpsimd", "scalar",
                  "sync"]
    return w_pieces, x_pieces, out_qs


def _build_nc():
    p = _params()
    mcols, p8, s8 = p["mcols"], p["p8"], p["s8"]
    mid_range, midw_range, woff = p["mid_range"], p["midw_range"], p["woff"]
    S = p["S"]
    w_pieces, x_pieces, out_qs = _dma_plan(p)

    nc = bacc.Bacc(None, target_bir_lowering=False)
    x8_d = nc.dram_tensor("x8", (2, 128, 2, MB2P), F8_DT, kind="ExternalInput")
    xr8_d = nc.dram_tensor("xr8", (2, 128, 2, MB2P), F8_DT,
                           kind="ExternalInput")
    w8_d = nc.dram_tensor("w8", (128, 2, S), F8_DT, kind="ExternalInput")
    out_d = nc.dram_tensor("out", (NTILE, 128, NROWS), OUT_DT,
                           kind="ExternalOutput")

    with ExitStack() as ctx:
        tc = ctx.enter_context(tile.TileContext(nc))
        xp = ctx.enter_context(tc.tile_pool(name="xp", bufs=1))
        wp = ctx.enter_context(tc.tile_pool(name="wp", bufs=1))
        op = ctx.enter_context(tc.tile_pool(name="op", bufs=1))
        pp = ctx.enter_context(tc.tile_pool(name="pp", bufs=1, space="PSUM"))

        # PSUM: one full bank per frame-tile + one warm-up scratch bank
        ps = [pp.tile([128, 512], mybir.dt.float32, name=f"ps{t}",
                      tag=f"ps{t}") for t in range(NTILE)]
        pw = pp.tile([128, 512], mybir.dt.float32, name="pw", tag="pw")

        warm_sb = xp.tile([128, max(WARM_N, 128)], MM_DT, name="warm",
                          tag="warm")
        nc.vector.memset(warm_sb[:].bitcast(mybir.dt.float32), 0.0)
        if int(_os.environ.get("K_ARM", "0")):
            for t in range(NTILE):
                nc.vector.memset(ps[t][:, 0:NROWS], 0.0)
        for _ in range(N_WARM):
            nc.tensor.matmul(pw[:, 0:WARM_N], warm_sb[:, 0:128],
                             warm_sb[:, 0:WARM_N],
                             start=True, stop=True, skip_group_check=True)

        # --- SBUF tiles + input DMA streams ---
        x8s = [xp.tile([128, 2, MB2P], F8_DT, name=f"x8_{u}", tag=f"x8_{u}")
               for u in range(2)]
        xr8s = [xp.tile([128, 2, MB2P], F8_DT, name=f"xr8_{u}",
                        tag=f"xr8_{u}") for u in range(2)]
        w8s = wp.tile([128, 2, S], F8_DT, name="w8_sb", tag="w8_sb")

        qs = {"sync": nc.sync, "scalar": nc.scalar, "gpsimd": nc.gpsimd,
              "vector": nc.vector}
        for q, tn, lo, hi in w_pieces:
            if hi > lo:
                qs[q].dma_start(w8s[:, :, lo:hi], w8_d[:, :, lo:hi])
        for q, tn, u, lo, hi in x_pieces:
            src = x8_d if tn == "x8" else xr8_d
            dst = x8s if tn == "x8" else xr8s
            qs[q].dma_start(dst[u][:, :, lo:hi], src[u][:, :, lo:hi])

        # --- matmul streams: one per frame-tile ---
        DR = mybir.MatmulPerfMode.DoubleRow

        def stream_emits(qlist_a, qlist_b):
            emits = []
            for q in qlist_a:           # phase A
                lo, hi = midw_range(q)
                if hi > lo:
                    emits.append(("midw", q, lo, hi, woff[("midw", q)]))
                if p8[q] > 0 and q != 0:
                    emits.append(("pre", q, 0, int(p8[q]),
                                  woff[("pre", q)]))
                if mcols[2 * q] > s8[q]:
                    emits.append(("suf", q, int(s8[q]), int(mcols[2 * q]),
                                  woff[("suf", q)]))
            for q in qlist_b:           # phase B
                lo, hi = mid_range(q)
                if hi > lo:
                    emits.append(("midr", q, lo, hi, woff[("midr", q)]))
                    # w8 * xr8 reuses the MIDW block columns
                    wlo, _ = midw_range(q)
                    emits.append(("xres", q, lo, hi,
                                  woff[("midw", q)] + (lo - wlo)))
            return emits

        def emit_mms(t, emits, first, last):
            v0, m = V0[t], MT[t]
            for n, (kind, q, lo, hi, o8) in enumerate(emits):
                u, jp = q % 2, q // 2
                stat = xr8s[u] if kind == "xres" else x8s[u]
                nc.tensor.matmul(
                    ps[t][0:m, lo:hi],
                    stat[:, :, v0 + jp:v0 + jp + m],
                    w8s[:, :, o8:o8 + hi - lo],
                    perf_mode=DR,
                    start=(first and n == 0),
                    stop=(last and n == len(emits) - 1),
                    skip_group_check=True)

        arm = p["arm_via_memset"]

        def emit_stream(t):
            emit_mms(t, stream_emits(range(NPAIR), range(NPAIR)),
                     not arm, True)

        ots = [op.tile([128, NROWS], OUT_DT, name=f"o{t}", tag=f"o{t}")
               for t in range(NTILE)]
        tail_split = int(_os.environ.get("K_TAILSPLIT", "0"))
        # bins 42..83 (cols 84:168) have windows <= pair 3; after pairs 0-3
        # their PSUM columns are final and can be copied mid-stream
        for t in range(NTILE):
            m = MT[t]
            if tail_split and t == NTILE - 1:
                emit_mms(t, stream_emits(range(4), range(4)), True, False)
                nc.vector.tensor_copy(ots[t][0:m, 84:NROWS],
                                      ps[t][0:m, 84:NROWS])
                emit_mms(t, stream_emits(range(4, NPAIR), range(4, NPAIR)),
                         False, True)
                nc.vector.tensor_copy(ots[t][0:m, 0:84], ps[t][0:m, 0:84])
            else:
                emit_stream(t)
                nc.vector.tensor_copy(ots[t][0:m, :], ps[t][0:m, 0:NROWS])
            qs[out_qs[t]].dma_start(out_d[t, 0:m, :], ots[t][0:m, :])
    nc.finalize()
    return nc


def get_nc():
    global _NC
    if _NC is None:
        _NC = _build_nc()
    return _NC


def _pack_x(xb):
    """(C, T) -> fp8 x8 (2, 128, 2, MB2P) + fp8 residual xr8 (same).

    x8[u, r, i, m] = fp8(xcat[m*512 + u*256 + i*128 + r]);
    xr8 = fp8(xcat - x8).  xcat = [ch0 blocks | ch1 blocks], zero tails."""
    xpad = np.zeros((C, MBLK * 512), dtype=np.float32)
    xpad[:, :T] = xb
    xcat = xpad.reshape(MB2, 512)
    xq = xcat.astype(F8_NP)
    xr = (xcat - xq.astype(np.float32)).astype(F8_NP)
    x8 = np.zeros((2, 128, 2, MB2P), dtype=F8_NP)
    xr8 = np.zeros((2, 128, 2, MB2P), dtype=F8_NP)
    x8[:, :, :, :MB2] = np.ascontiguousarray(
        xq.view(np.uint8).reshape(MB2, 2, 2, 128).transpose(1, 3, 2, 0)
    ).view(F8_NP)
    xr8[:, :, :, :MB2] = np.ascontiguousarray(
        xr.view(np.uint8).reshape(MB2, 2, 2, 128).transpose(1, 3, 2, 0)
    ).view(F8_NP)
    return x8, xr8


def _core_inputs(xb):
    p = _params()
    x8, xr8 = _pack_x(xb)
    return {"x8": x8, "xr8": xr8, "w8": p["w8"]}


def kernel(x):
    global LAST_RESULTS
    x = np.asarray(x, dtype=np.float32)
    assert x.shape == (B, C, T)
    in_maps = [_core_inputs(x[b]) for b in range(B)]
    nc = get_nc()
    res = run_bass_kernel_spmd(nc, in_maps, core_ids=list(range(N_CORES)))
    LAST_RESULTS = res
    out = np.empty((B, C, N_BINS, F, 2), dtype=np.float32)
    for b in range(B):
        raw = np.asarray(res.results[b]["out"])  # (NTILE, 128, NROWS)
        out[b] = _unpack_out(raw)
    return out


def _unpack_out(raw):
    """(NTILE, 128, NROWS) -> (C, N_BINS, F, 2)."""
    raw = np.asarray(raw, dtype=np.float32)
    cat = raw.reshape(NTILE * 128, NROWS)[:V0[-1] + MT[-1]]  # (840, 168)
    o = np.empty((C, N_BINS, F, 2), dtype=np.float32)
    o[0] = cat[0:F].reshape(F, N_BINS, 2).transpose(1, 0, 2)
    o[1] = cat[MBLK:MBLK + F].reshape(F, N_BINS, 2).transpose(1, 0, 2)
    return o
